# revision 29
# baseline (speedup 1.0000x reference)
"""Trainium2 Bass kernel for the CorefSeq segment-reduce problem.

Computes, for batch b:
  o[b] = concat([mean of emb[b,s] over s where mentions[b,s]==l for l in (2,3,4)])
  out[b] = relu(o[b] @ W1 + b1) @ W2 + b2

Sharding: data-parallel over the batch axis across 8 NeuronCores
(128 batches per core); classifier weights replicated.

Memory-regime optimization: only tokens labeled 2/3/4 (~60%) contribute
to the output, so instead of streaming all 201MB of the per-core
embeddings slice, the kernel row-gathers just the contributing rows
(~121MB) with dma_gather (one 3KB descriptor per row).

Per batch-pair (2 batches, one gather call) the host prepares:
  - a compacted int16 row list: batch-0 label-2 tokens, then label-3,
    label-4, then batch-1's, padded to a uniform count T_FIX with
    duplicate rows (weight 0) and then -1 (trimmed by the DMA engine);
  - a [128, 6, 6] weight matrix masksG: slot (tile*128+p) of the
    gathered stream carries 1/count(b,l) in the column of its (b,l)
    segment, 0 elsewhere (so the matmuls below emit means directly).
The device executes all FLOPs and all heavy data movement:
  - per pair: dma_gather -> 6 accumulating f32r matmuls
    (lhsT = masksG tile [128,6], moving = gathered rows [128,768])
    write the 6 means into PSUM at column-tile base 32*(pair%4):
    one [128,768] PSUM tile holds 4 pairs = 8 batches;
  - per 4-pair group: ScalarE evacuates PSUM->SBUF, 6 stacked TensorE
    transposes + DVE copies scatter into the feature-major o^T [2304,b]
    activation matrix (6 transposes per 8 batches, not 6 per batch);
  - one batched MLP over all 128 b at the end (feature-major matmuls).
"""

import sys

import numpy as np

if "/opt/trn_rl_repo" not in sys.path:
    sys.path.insert(0, "/opt/trn_rl_repo")

import concourse.bacc as bacc
import concourse.bass as bass
import concourse.mybir as mybir
import concourse.tile as tile
from concourse.bass_utils import run_bass_kernel_spmd
from concourse.masks import make_identity

N_CORES = 8
B, S, H = 1024, 512, 768
HC = H // 128  # 6 h-chunks of 128 (transpose tiles)
NCLS = 3       # labels (2,3,4) and also output classes
F = NCLS * H   # 2304 concat features
FC = F // 128  # 18
J = 512        # hidden dim
JC = J // 128  # 4
PB = 2         # batches per gather call (pair)
NSEG = PB * NCLS          # 6 stream segments per pair
SLOTS = 768               # static slot budget per pair (6 tiles of 128)
TILES = SLOTS // 128      # 6
GP = 16                   # pairs per transpose group (6*16=96 partitions)

MM_DT = mybir.dt.float32r
_LAST = {}


def _build(nb: int, t_fix: int) -> bass.Bass:
    # 4 SWDGE queues + a 2x descriptor-ring carveout: each ~650-row gather
    # pushes ~1350 descriptors; with the default 16KB carveout (1024 descs
    # per queue ring) the Q7 generator blocks in await_space until the
    # previous call drains, serializing DGE with the SDMA engines.
    nc = bacc.Bacc(
        trn_type="TRN2", num_swdge_queues=4, dynamic_dma_scratch_size=32768
    )
    f32 = mybir.dt.float32
    npairs = nb // PB
    tiles_used = (t_fix + 127) // 128  # gathered 128-slot tiles with data

    emb = nc.dram_tensor("embeddings", [nb, S, H], f32, kind="ExternalInput")
    idxd = nc.dram_tensor(
        "gidx", [128, npairs, SLOTS // 16], mybir.dt.int16, kind="ExternalInput"
    )
    mgd = nc.dram_tensor(
        "masksG", [128, npairs, TILES, NSEG], f32, kind="ExternalInput"
    )
    w1 = nc.dram_tensor("W1", [F, J], f32, kind="ExternalInput")
    b1 = nc.dram_tensor("b1", [J], f32, kind="ExternalInput")
    w2 = nc.dram_tensor("W2", [J, NCLS], f32, kind="ExternalInput")
    b2 = nc.dram_tensor("b2", [NCLS], f32, kind="ExternalInput")
    out = nc.dram_tensor("out", [nb, NCLS], f32, kind="ExternalOutput")

    EMB_BUFS = 5

    with tile.TileContext(nc) as tc:
        with (
            tc.tile_pool(name="consts", bufs=1) as consts,
            tc.tile_pool(name="embp", bufs=1) as embp,
            tc.tile_pool(name="osp", bufs=1) as osp,
            tc.tile_pool(name="pairp", bufs=3) as pairp,
            tc.tile_pool(name="psmean", bufs=2, space="PSUM") as psmean,
            tc.tile_pool(name="pssmall", bufs=2, space="PSUM") as pssmall,
        ):
            # identity: gpsimd builds it, DVE re-copies it so its last producer
            # is DVE (single-semaphore waits for PE transposes).
            ident_g = consts.tile([128, 128], f32)
            make_identity(nc, ident_g)
            ident = consts.tile([128, 128], f32)
            nc.vector.tensor_copy(out=ident, in_=ident_g)

            # gather indices + per-slot weight matrices (host-prepared)
            idxs = consts.tile([128, npairs, SLOTS // 16], mybir.dt.int16)
            nc.sync.dma_start(out=idxs, in_=idxd[:, :, :])
            mG = consts.tile([128, npairs, TILES, NSEG], MM_DT)
            nc.sync.dma_start(
                out=mG.rearrange("p j t m -> p j (t m)"),
                in_=mgd.rearrange("p j t m -> p j (t m)").bitcast(MM_DT),
            )

            # ---- classifier weights (feature-major layouts) ----
            w1sb = consts.tile([128, FC, J], f32)
            nc.sync.dma_start(out=w1sb, in_=w1.rearrange("(kc k) j -> k kc j", k=128))
            b1T = consts.tile([128, JC], f32)
            nc.sync.dma_start(out=b1T, in_=b1.rearrange("(jc j) -> j jc", j=128))
            w2sb = consts.tile([128, JC, NCLS], f32)
            nc.sync.dma_start(out=w2sb, in_=w2.rearrange("(jc j) m -> j jc m", j=128))
            b2T = consts.tile([NCLS, 1], f32)
            nc.sync.dma_start(out=b2T, in_=b2.rearrange("(m one) -> m one", one=1))

            # o^T[feature, b] activation matrix for the MLP
            oT = consts.tile([128, NCLS, HC, 128], f32)

            # gather destinations: cycled manually (not pool-cycled) so the
            # one-time zeroing below is visible to the dep tracker. Slots
            # >= T_FIX keep stale-but-finite data; their weights are 0.
            # (DVE memset can't emit f32r; a broadcast copy from a zeroed f32
            # column is a verifier-approved f32r producer.)
            zrow = consts.tile([128, 1], f32)
            nc.vector.memset(zrow, 0.0)
            embts = []
            for i in range(EMB_BUFS):
                t = embp.tile([128, tiles_used * H], MM_DT, tag=f"embt{i}")
                nc.vector.tensor_copy(
                    out=t, in_=zrow.to_broadcast([128, tiles_used * H])
                )
                embts.append(t)

            # double-buffered transpose-stack tiles, zeroed once. Each pair's
            # 6 mean-rows are DMA'd (SBUF->SBUF moves across partitions) to
            # partitions 6*(pair%16)+[0,6).
            stacks = []
            for i in range(2):
                s = osp.tile([128, H], f32, tag=f"stack{i}")
                nc.vector.memset(s, 0.0)
                stacks.append(s)

            oTr = oT.rearrange("p l h (bq two) -> p l h bq two", two=PB)

            # ---- main loop: gather rows, segment-mean via matmul ----
            for j in range(npairs):
                g, q = divmod(j, GP)
                gsz = min(GP, npairs - g * GP)
                emb_t = embts[j % EMB_BUFS].rearrange(
                    "p (t h) -> p t h", h=H
                )
                nc.gpsimd.dma_gather(
                    out_ap=emb_t[:, :, :],
                    in_ap=emb[j * PB : (j + 1) * PB]
                    .rearrange("b s h -> (b s) h")
                    .bitcast(MM_DT),
                    idxs_ap=idxs[:, j, : t_fix // 16],
                    num_idxs=t_fix,
                    num_idxs_reg=t_fix,
                    elem_size=H,
                    queue_num=j % 4,
                )
                ps_pair = psmean.tile([NSEG, H], f32, tag="pair")
                for t in range(tiles_used):
                    lhsT = mG[:, j, t, :]
                    rhs = emb_t[:, t, :]
                    nc.tensor.matmul(
                        ps_pair[:, 0:512], lhsT, rhs[:, 0:512],
                        start=(t == 0), stop=(t == tiles_used - 1),
                    )
                    nc.tensor.matmul(
                        ps_pair[:, 512:H], lhsT, rhs[:, 512:H],
                        start=(t == 0), stop=(t == tiles_used - 1),
                    )
                pairS = pairp.tile([NSEG, H], f32, tag="pairS")
                nc.scalar.copy(out=pairS, in_=ps_pair)
                stackS = stacks[g % 2]
                nc.sync.dma_start(
                    out=stackS[NSEG * q : NSEG * (q + 1), :], in_=pairS
                )
                if q == gsz - 1:
                    # 6 stacked transposes scatter the group's 32 batches
                    # of means into oT[l*768 + hc*128 + p, b]
                    np_ = NSEG * gsz  # used partitions
                    for hc in range(HC):
                        ps_t = pssmall.tile([128, NSEG * GP], f32, tag="ptr")
                        nc.tensor.transpose(
                            ps_t[:, :np_],
                            stackS[:np_, hc * 128 : (hc + 1) * 128],
                            ident[:np_, :np_],
                        )
                        ps_tv = ps_t.rearrange("p (q r) -> p r q", q=GP)
                        for bb in range(PB):
                            nc.vector.tensor_copy(
                                out=oTr[:, :, hc, GP * g : GP * g + gsz, bb],
                                in_=ps_tv[:, NCLS * bb : NCLS * (bb + 1), :gsz],
                            )

            # ---- MLP over all b at once (feature-major) ----
            hT = consts.tile([128, JC, 128], f32)
            for jc in range(JC):
                ps_h = pssmall.tile([128, 128], f32, tag="small")
                for kc in range(FC):
                    nc.tensor.matmul(
                        ps_h[:, :nb],
                        w1sb[:, kc, jc * 128 : (jc + 1) * 128],
                        oT[:, kc // HC, kc % HC, :nb],
                        start=(kc == 0), stop=(kc == FC - 1),
                    )
                nc.scalar.activation(
                    out=hT[:, jc, :nb], in_=ps_h[:, :nb],
                    func=mybir.ActivationFunctionType.Relu,
                    bias=b1T[:, jc : jc + 1], scale=1.0,
                )
            ps_o = pssmall.tile([NCLS, 128], f32, tag="small")
            for jc in range(JC):
                nc.tensor.matmul(
                    ps_o[:, :nb], w2sb[:, jc, :], hT[:, jc, :nb],
                    start=(jc == 0), stop=(jc == JC - 1),
                )
            outT = consts.tile([NCLS, 128], f32)
            nc.vector.tensor_scalar_add(
                out=outT[:, :nb], in0=ps_o[:, :nb], scalar1=b2T[:, 0:1]
            )
            ps_ob = pssmall.tile([128, NCLS], f32, tag="small")
            nc.tensor.transpose(ps_ob[:nb], outT[:, :nb], ident[:NCLS, :NCLS])
            outB = consts.tile([128, NCLS], f32)
            nc.vector.tensor_copy(out=outB[:nb], in_=ps_ob[:nb])
            nc.sync.dma_start(out=out[:, :], in_=outB[:nb])

    if not nc.is_finalized():
        nc.finalize()  # Bacc: reg alloc + semaphore-wait splitting
    return nc


def _prep_streams(ment: np.ndarray):
    """Compacted per-pair gather rows + per-slot weight matrices.

    ment: [NB, S] int labels for one core. Returns (idx_wrapped, masksG,
    t_max) where idx entries are rows local to the pair's 2*S-row window.
    """
    nbl, _ = ment.shape
    npairs = nbl // PB
    idx_arr = np.full((npairs, SLOTS), -1, dtype=np.int16)
    seg_arr = np.zeros((npairs, SLOTS), dtype=np.int16)
    w_arr = np.zeros((npairs, SLOTS), dtype=np.float32)
    t_list = np.zeros(npairs, dtype=np.int64)
    for j in range(npairs):
        pos = 0
        for bb in range(PB):
            row = ment[j * PB + bb]
            for l in range(NCLS):
                (s_sel,) = np.nonzero(row == l + 2)
                n = len(s_sel)
                idx_arr[j, pos : pos + n] = bb * S + s_sel
                seg_arr[j, pos : pos + n] = bb * NCLS + l
                w_arr[j, pos : pos + n] = 1.0 / n
                pos += n
        t_list[j] = pos
    return idx_arr, seg_arr, w_arr, t_list


def kernel(embeddings, mentions, W1, b1, W2, b2):
    emb = np.asarray(embeddings, dtype=np.float32)
    ment = np.asarray(mentions).astype(np.int64)
    w1 = np.ascontiguousarray(np.asarray(W1, dtype=np.float32))
    b1a = np.ascontiguousarray(np.asarray(b1, dtype=np.float32))
    w2 = np.ascontiguousarray(np.asarray(W2, dtype=np.float32))
    b2a = np.ascontiguousarray(np.asarray(b2, dtype=np.float32))

    nb = B // N_CORES
    npairs = nb // PB

    per_core = [
        _prep_streams(ment[i * nb : (i + 1) * nb]) for i in range(N_CORES)
    ]
    t_max = int(max(t.max() for _, _, _, t in per_core))
    assert t_max <= SLOTS, f"pair stream overflow: {t_max} > {SLOTS}"
    t_fix = min((t_max + 15) // 16 * 16, SLOTS)

    nc = _build(nb, t_fix)
    in_maps = []
    for i in range(N_CORES):
        idx_arr, seg_arr, w_arr, t_list = per_core[i]
        # pad each pair's stream to exactly t_fix valid entries (dup row 0
        # with weight 0), then trailing -1 which the DMA engine trims.
        for j in range(npairs):
            t = int(t_list[j])
            idx_arr[j, t:t_fix] = 0
            w_arr[j, t:t_fix] = 0.0
            idx_arr[j, t_fix:] = -1
        # wrap by 16 (position i -> [i%16, i//16]) and replicate to 128 rows
        idxw16 = np.ascontiguousarray(
            idx_arr.reshape(npairs, SLOTS // 16, 16).transpose(2, 0, 1)
        )  # [16, npairs, SLOTS//16]
        idxw = np.ascontiguousarray(np.tile(idxw16, (8, 1, 1)))
        # masksG[p, j, tile, m] = w[slot] * (seg[slot] == m), slot = tile*128+p
        seg_t = seg_arr.reshape(npairs, TILES, 128)
        w_t = w_arr.reshape(npairs, TILES, 128)
        mg = (
            (seg_t[:, :, :, None] == np.arange(NSEG)[None, None, None, :])
            * w_t[:, :, :, None]
        ).astype(np.float32)  # [npairs, TILES, 128, NSEG]
        mg = np.ascontiguousarray(mg.transpose(2, 0, 1, 3))  # [128, npairs, T, M]

        sl = slice(i * nb, (i + 1) * nb)
        in_maps.append(
            {
                "embeddings": np.ascontiguousarray(emb[sl]),
                "gidx": idxw,
                "masksG": mg,
                "W1": w1, "b1": b1a, "W2": w2, "b2": b2a,
            }
        )
    res = run_bass_kernel_spmd(nc, in_maps, core_ids=list(range(N_CORES)))
    _LAST["exec_time_ns"] = res.exec_time_ns
    _LAST["result"] = res
    return np.concatenate([res.results[i]["out"] for i in range(N_CORES)], axis=0)


# revision 39
# speedup vs baseline: 1.0624x; 1.0624x over previous
"""Trainium2 Bass kernel for the CorefSeq segment-reduce problem.

Computes, for batch b:
  o[b] = concat([mean of emb[b,s] over s where mentions[b,s]==l for l in (2,3,4)])
  out[b] = relu(o[b] @ W1 + b1) @ W2 + b2

Sharding: data-parallel over the batch axis across 8 NeuronCores
(128 batches per core); classifier weights replicated.

Memory-regime optimization: only tokens labeled 2/3/4 (~60%) contribute
to the output, so instead of streaming all 201MB of the per-core
embeddings slice, the kernel row-gathers just the contributing rows
(~121MB) with dma_gather (one 3KB descriptor per row).

Per batch-pair (2 batches, one gather call) the host prepares:
  - a compacted int16 row list: batch-0 label-2 tokens, then label-3,
    label-4, then batch-1's, padded to a uniform count T_FIX with
    duplicate rows (weight 0) and then -1 (trimmed by the DMA engine);
  - a [128, 6, 6] weight matrix masksG: slot (tile*128+p) of the
    gathered stream carries 1/count(b,l) in the column of its (b,l)
    segment, 0 elsewhere (so the matmuls below emit means directly).
The device executes all FLOPs and all heavy data movement:
  - per pair: dma_gather -> 6 accumulating f32r matmuls
    (lhsT = masksG tile [128,6], moving = gathered rows [128,768])
    write the 6 means into PSUM at column-tile base 32*(pair%4):
    one [128,768] PSUM tile holds 4 pairs = 8 batches;
  - per 4-pair group: ScalarE evacuates PSUM->SBUF, 6 stacked TensorE
    transposes + DVE copies scatter into the feature-major o^T [2304,b]
    activation matrix (6 transposes per 8 batches, not 6 per batch);
  - one batched MLP over all 128 b at the end (feature-major matmuls).
"""

import sys

import numpy as np

if "/opt/trn_rl_repo" not in sys.path:
    sys.path.insert(0, "/opt/trn_rl_repo")

import concourse.bacc as bacc
import concourse.bass as bass
import concourse.mybir as mybir
import concourse.tile as tile
from concourse.bass_utils import run_bass_kernel_spmd
from concourse.masks import make_identity

N_CORES = 8
B, S, H = 1024, 512, 768
HC = H // 128  # 6 h-chunks of 128 (transpose tiles)
NCLS = 3       # labels (2,3,4) and also output classes
F = NCLS * H   # 2304 concat features
FC = F // 128  # 18
J = 512        # hidden dim
JC = J // 128  # 4
PB = 2         # batches per gather call (pair)
NSEG = PB * NCLS          # 6 stream segments per pair
SLOTS = 768               # static slot budget per pair (6 tiles of 128)
TILES = SLOTS // 128      # 6
DTILES = PB * S // 128    # 8 s-chunks when a pair is streamed densely
GP = 16                   # pairs per transpose group (6*16=96 partitions)
DENSE_EVERY = 8           # every k-th pair streams densely via HWDGE:
                          # balances Pool-engine descriptor generation
                          # (~6.5us per gather) against DMA bytes

MM_DT = mybir.dt.float32r
_LAST = {}


def _build(nb: int, t_fix: int) -> bass.Bass:
    # 4 SWDGE queues + a 2x descriptor-ring carveout: each ~650-row gather
    # pushes ~1350 descriptors; with the default 16KB carveout (1024 descs
    # per queue ring) the Q7 generator blocks in await_space until the
    # previous call drains, serializing DGE with the SDMA engines.
    nc = bacc.Bacc(trn_type="TRN2", num_swdge_queues=4)
    f32 = mybir.dt.float32
    npairs = nb // PB
    tiles_used = (t_fix + 127) // 128  # gathered 128-slot tiles with data

    emb = nc.dram_tensor("embeddings", [nb, S, H], f32, kind="ExternalInput")
    idxd = nc.dram_tensor(
        "gidx", [128, npairs, SLOTS // 16], mybir.dt.int16, kind="ExternalInput"
    )
    mgd = nc.dram_tensor(
        "masksG", [128, npairs, TILES, NSEG], f32, kind="ExternalInput"
    )
    nd = len([j for j in range(npairs) if j % DENSE_EVERY == DENSE_EVERY - 1])
    mdd = nc.dram_tensor(
        "masksD", [128, max(nd, 1), DTILES, NSEG], f32, kind="ExternalInput"
    )
    w1 = nc.dram_tensor("W1", [F, J], f32, kind="ExternalInput")
    b1 = nc.dram_tensor("b1", [J], f32, kind="ExternalInput")
    w2 = nc.dram_tensor("W2", [J, NCLS], f32, kind="ExternalInput")
    b2 = nc.dram_tensor("b2", [NCLS], f32, kind="ExternalInput")
    out = nc.dram_tensor("out", [nb, NCLS], f32, kind="ExternalOutput")

    EMB_BUFS = 5

    with tile.TileContext(nc) as tc:
        with (
            tc.tile_pool(name="consts", bufs=1) as consts,
            tc.tile_pool(name="embp", bufs=1) as embp,
            tc.tile_pool(name="osp", bufs=1) as osp,
            tc.tile_pool(name="pairp", bufs=3) as pairp,
            tc.tile_pool(name="psmean", bufs=2, space="PSUM") as psmean,
            tc.tile_pool(name="pssmall", bufs=2, space="PSUM") as pssmall,
        ):
            # identity: gpsimd builds it, DVE re-copies it so its last producer
            # is DVE (single-semaphore waits for PE transposes).
            ident_g = consts.tile([128, 128], f32)
            make_identity(nc, ident_g)
            ident = consts.tile([128, 128], f32)
            nc.vector.tensor_copy(out=ident, in_=ident_g)

            # gather indices + per-slot weight matrices (host-prepared)
            idxs = consts.tile([128, npairs, SLOTS // 16], mybir.dt.int16)
            nc.sync.dma_start(out=idxs, in_=idxd[:, :, :])
            mG = consts.tile([128, npairs, TILES, NSEG], MM_DT)
            nc.sync.dma_start(
                out=mG.rearrange("p j t m -> p j (t m)"),
                in_=mgd.rearrange("p j t m -> p j (t m)").bitcast(MM_DT),
            )
            mD = consts.tile([128, max(nd, 1), DTILES, NSEG], MM_DT)
            nc.sync.dma_start(
                out=mD.rearrange("p j t m -> p j (t m)"),
                in_=mdd.rearrange("p j t m -> p j (t m)").bitcast(MM_DT),
            )

            # ---- classifier weights (feature-major layouts) ----
            w1sb = consts.tile([128, FC, J], f32)
            nc.sync.dma_start(out=w1sb, in_=w1.rearrange("(kc k) j -> k kc j", k=128))
            b1T = consts.tile([128, JC], f32)
            nc.sync.dma_start(out=b1T, in_=b1.rearrange("(jc j) -> j jc", j=128))
            w2sb = consts.tile([128, JC, NCLS], f32)
            nc.sync.dma_start(out=w2sb, in_=w2.rearrange("(jc j) m -> j jc m", j=128))
            b2T = consts.tile([NCLS, 1], f32)
            nc.sync.dma_start(out=b2T, in_=b2.rearrange("(m one) -> m one", one=1))

            # o^T[feature, b] activation matrix for the MLP
            oT = consts.tile([128, NCLS, HC, 128], f32)

            # gather destinations: cycled manually (not pool-cycled) so the
            # one-time zeroing below is visible to the dep tracker. Slots
            # >= T_FIX keep stale-but-finite data; their weights are 0.
            # (DVE memset can't emit f32r; a broadcast copy from a zeroed f32
            # column is a verifier-approved f32r producer.)
            zrow = consts.tile([128, 1], f32)
            nc.vector.memset(zrow, 0.0)
            embts = []
            for i in range(EMB_BUFS):
                t = embp.tile([128, DTILES * H], MM_DT, tag=f"embt{i}")
                nc.vector.tensor_copy(
                    out=t, in_=zrow.to_broadcast([128, DTILES * H])
                )
                embts.append(t)

            # double-buffered transpose-stack tiles, zeroed once. Each pair's
            # 6 mean-rows are DMA'd (SBUF->SBUF moves across partitions) to
            # partitions 6*(pair%16)+[0,6).
            stacks = []
            for i in range(2):
                s = osp.tile([128, H], f32, tag=f"stack{i}")
                nc.vector.memset(s, 0.0)
                stacks.append(s)

            oTr = oT.rearrange("p l h (bq two) -> p l h bq two", two=PB)

            # ---- main loop: gather rows, segment-mean via matmul ----
            di = 0
            gi = 0  # gather counter: keeps queue_num in lockstep with the
            # scheduler's round-robin DMASW semaphore-lane assignment
            for j in range(npairs):
                g, q = divmod(j, GP)
                gsz = min(GP, npairs - g * GP)
                dense = j % DENSE_EVERY == DENSE_EVERY - 1
                emb_t = embts[j % EMB_BUFS].rearrange(
                    "p (t h) -> p t h", h=H
                )
                if dense:
                    nc.sync.dma_start(
                        out=emb_t[:, :, :],
                        in_=emb[j * PB : (j + 1) * PB]
                        .rearrange("bb (c p) h -> p (bb c) h", p=128)
                        .bitcast(MM_DT),
                    )
                    ntile, mW, wj = DTILES, mD, di
                    di += 1
                else:
                    nc.gpsimd.dma_gather(
                        out_ap=emb_t[:, :tiles_used, :],
                        in_ap=emb[j * PB : (j + 1) * PB]
                        .rearrange("b s h -> (b s) h")
                        .bitcast(MM_DT),
                        idxs_ap=idxs[:, j, : t_fix // 16],
                        num_idxs=t_fix,
                        num_idxs_reg=t_fix,
                        elem_size=H,
                        queue_num=gi % 4,
                        single_packet=False,
                    )
                    gi += 1
                    ntile, mW, wj = tiles_used, mG, j
                ps_pair = psmean.tile([NSEG, H], f32, tag="pair")
                for t in range(ntile):
                    lhsT = mW[:, wj, t, :]
                    rhs = emb_t[:, t, :]
                    nc.tensor.matmul(
                        ps_pair[:, 0:512], lhsT, rhs[:, 0:512],
                        start=(t == 0), stop=(t == ntile - 1),
                    )
                    nc.tensor.matmul(
                        ps_pair[:, 512:H], lhsT, rhs[:, 512:H],
                        start=(t == 0), stop=(t == ntile - 1),
                    )
                pairS = pairp.tile([NSEG, H], f32, tag="pairS")
                nc.scalar.copy(out=pairS, in_=ps_pair)
                stackS = stacks[g % 2]
                nc.sync.dma_start(
                    out=stackS[NSEG * q : NSEG * (q + 1), :], in_=pairS
                )
                if q == gsz - 1:
                    # 6 stacked transposes scatter the group's 32 batches
                    # of means into oT[l*768 + hc*128 + p, b]
                    np_ = NSEG * gsz  # used partitions
                    for hc in range(HC):
                        ps_t = pssmall.tile([128, NSEG * GP], f32, tag="ptr")
                        nc.tensor.transpose(
                            ps_t[:, :np_],
                            stackS[:np_, hc * 128 : (hc + 1) * 128],
                            ident[:np_, :np_],
                        )
                        ps_tv = ps_t.rearrange("p (q r) -> p r q", q=GP)
                        for bb in range(PB):
                            nc.vector.tensor_copy(
                                out=oTr[:, :, hc, GP * g : GP * g + gsz, bb],
                                in_=ps_tv[:, NCLS * bb : NCLS * (bb + 1), :gsz],
                            )

            # ---- MLP over all b at once (feature-major) ----
            hT = consts.tile([128, JC, 128], f32)
            for jc in range(JC):
                ps_h = pssmall.tile([128, 128], f32, tag="small")
                for kc in range(FC):
                    nc.tensor.matmul(
                        ps_h[:, :nb],
                        w1sb[:, kc, jc * 128 : (jc + 1) * 128],
                        oT[:, kc // HC, kc % HC, :nb],
                        start=(kc == 0), stop=(kc == FC - 1),
                    )
                nc.scalar.activation(
                    out=hT[:, jc, :nb], in_=ps_h[:, :nb],
                    func=mybir.ActivationFunctionType.Relu,
                    bias=b1T[:, jc : jc + 1], scale=1.0,
                )
            ps_o = pssmall.tile([NCLS, 128], f32, tag="small")
            for jc in range(JC):
                nc.tensor.matmul(
                    ps_o[:, :nb], w2sb[:, jc, :], hT[:, jc, :nb],
                    start=(jc == 0), stop=(jc == JC - 1),
                )
            outT = consts.tile([NCLS, 128], f32)
            nc.vector.tensor_scalar_add(
                out=outT[:, :nb], in0=ps_o[:, :nb], scalar1=b2T[:, 0:1]
            )
            ps_ob = pssmall.tile([128, NCLS], f32, tag="small")
            nc.tensor.transpose(ps_ob[:nb], outT[:, :nb], ident[:NCLS, :NCLS])
            outB = consts.tile([128, NCLS], f32)
            nc.vector.tensor_copy(out=outB[:nb], in_=ps_ob[:nb])
            nc.sync.dma_start(out=out[:, :], in_=outB[:nb])

    if not nc.is_finalized():
        nc.finalize()  # Bacc: reg alloc + semaphore-wait splitting
    return nc


def _prep_dense(ment: np.ndarray):
    """Dense s-aligned weight matrices for the HWDGE-streamed pairs.

    Returns [128, nd, DTILES, NSEG]: tile t = bb*4+c covers token
    s = c*128+p of batch bb; column m = bb*3+l holds 1/count(b,l) where
    the token's label is l+2.
    """
    nbl, _ = ment.shape
    npairs = nbl // PB
    dense_js = [j for j in range(npairs) if j % DENSE_EVERY == DENSE_EVERY - 1]
    md = np.zeros((128, max(len(dense_js), 1), DTILES, NSEG), dtype=np.float32)
    for di, j in enumerate(dense_js):
        for bb in range(PB):
            row = ment[j * PB + bb]  # [S]
            for l in range(NCLS):
                mask = (row == l + 2).astype(np.float32)
                w = mask / mask.sum()
                for c in range(S // 128):
                    md[:, di, bb * (S // 128) + c, bb * NCLS + l] = w[
                        c * 128 : (c + 1) * 128
                    ]
    return md


def _prep_streams(ment: np.ndarray):
    """Compacted per-pair gather rows + per-slot weight matrices.

    ment: [NB, S] int labels for one core. Returns (idx_wrapped, masksG,
    t_max) where idx entries are rows local to the pair's 2*S-row window.
    """
    nbl, _ = ment.shape
    npairs = nbl // PB
    idx_arr = np.full((npairs, SLOTS), -1, dtype=np.int16)
    seg_arr = np.zeros((npairs, SLOTS), dtype=np.int16)
    w_arr = np.zeros((npairs, SLOTS), dtype=np.float32)
    t_list = np.zeros(npairs, dtype=np.int64)
    for j in range(npairs):
        pos = 0
        for bb in range(PB):
            row = ment[j * PB + bb]
            for l in range(NCLS):
                (s_sel,) = np.nonzero(row == l + 2)
                n = len(s_sel)
                idx_arr[j, pos : pos + n] = bb * S + s_sel
                seg_arr[j, pos : pos + n] = bb * NCLS + l
                w_arr[j, pos : pos + n] = 1.0 / n
                pos += n
        t_list[j] = pos
    return idx_arr, seg_arr, w_arr, t_list


def kernel(embeddings, mentions, W1, b1, W2, b2):
    emb = np.asarray(embeddings, dtype=np.float32)
    ment = np.asarray(mentions).astype(np.int64)
    w1 = np.ascontiguousarray(np.asarray(W1, dtype=np.float32))
    b1a = np.ascontiguousarray(np.asarray(b1, dtype=np.float32))
    w2 = np.ascontiguousarray(np.asarray(W2, dtype=np.float32))
    b2a = np.ascontiguousarray(np.asarray(b2, dtype=np.float32))

    nb = B // N_CORES
    npairs = nb // PB

    per_core = [
        _prep_streams(ment[i * nb : (i + 1) * nb]) for i in range(N_CORES)
    ]
    t_max = int(max(t.max() for _, _, _, t in per_core))
    assert t_max <= SLOTS, f"pair stream overflow: {t_max} > {SLOTS}"
    t_fix = min((t_max + 15) // 16 * 16, SLOTS)

    nc = _build(nb, t_fix)
    in_maps = []
    for i in range(N_CORES):
        idx_arr, seg_arr, w_arr, t_list = per_core[i]
        # pad each pair's stream to exactly t_fix valid entries (dup row 0
        # with weight 0), then trailing -1 which the DMA engine trims.
        for j in range(npairs):
            t = int(t_list[j])
            idx_arr[j, t:t_fix] = 0
            w_arr[j, t:t_fix] = 0.0
            idx_arr[j, t_fix:] = -1
        # wrap by 16 (position i -> [i%16, i//16]) and replicate to 128 rows
        idxw16 = np.ascontiguousarray(
            idx_arr.reshape(npairs, SLOTS // 16, 16).transpose(2, 0, 1)
        )  # [16, npairs, SLOTS//16]
        idxw = np.ascontiguousarray(np.tile(idxw16, (8, 1, 1)))
        # masksG[p, j, tile, m] = w[slot] * (seg[slot] == m), slot = tile*128+p
        seg_t = seg_arr.reshape(npairs, TILES, 128)
        w_t = w_arr.reshape(npairs, TILES, 128)
        mg = (
            (seg_t[:, :, :, None] == np.arange(NSEG)[None, None, None, :])
            * w_t[:, :, :, None]
        ).astype(np.float32)  # [npairs, TILES, 128, NSEG]
        mg = np.ascontiguousarray(mg.transpose(2, 0, 1, 3))  # [128, npairs, T, M]

        sl = slice(i * nb, (i + 1) * nb)
        in_maps.append(
            {
                "embeddings": np.ascontiguousarray(emb[sl]),
                "gidx": idxw,
                "masksG": mg,
                "masksD": _prep_dense(ment[sl]),
                "W1": w1, "b1": b1a, "W2": w2, "b2": b2a,
            }
        )
    res = run_bass_kernel_spmd(nc, in_maps, core_ids=list(range(N_CORES)))
    _LAST["exec_time_ns"] = res.exec_time_ns
    _LAST["result"] = res
    return np.concatenate([res.results[i]["out"] for i in range(N_CORES)], axis=0)
